# revision 41
# baseline (speedup 1.0000x reference)
"""Trainium2 Bass kernel for nn_C3S_RegularLoss.

reference:
    xr = x.reshape(B, P, D); xn = xr / ||xr||_2(axis=-1)
    s = mean_b(xn)                     # (P, D)
    corr = s @ s.T                     # (P, P)
    loss = (sum(corr) - 3*trace(corr) + 2P) / 2 * gamma

Reformulated without the corr matrix, with S = sum_b xn (sum, not mean):
    loss = ((A - 3*B2) / B^2 + 2P) / 2 * gamma,
    A = ||t||^2, t = sum_p S_p, B2 = sum_p ||S_p||^2

Sharding: data-parallel over the batch dim, 8 cores x 1024 rows.

Per core:
  - stream x as 8 row-tiles of (128, 8192), SWDGE DMA with fp32->bf16
    cast in flight (HBM fp32 read side is the ~94us roofline); first and
    last tiles split per part so their compute chains pipeline at part
    granularity
  - per-row sum-of-squares: parts 0/1 on ACT (Square + free accum),
    parts 2/3 on DVE (tensor_mul + tensor_reduce; the fused
    tensor_tensor_reduce wedges this runtime) -- neither engine
    saturates, so the final tile's chain is short
  - S_p = sum_b x_b,p / ||x_b,p|| via PE: stationary r = 1/norm
    [128,1] per part, PSUM accumulation (part p at partition 32p).
    Two accumulators: S_a = tile 0 only, AllReduce'd mid-stream (the
    in-flight collective also equalizes per-rank stream pace -- an
    input-independent warmup does not, and the final AR then eats
    ~10us of rank skew); S_b = tiles 1..7, AllReduce'd at the end.
  - endgame: evac S_b (ACT/DVE halves into separate tiles), 2 DMAs ->
    cc_in_b, AR2, load both AR outputs as bf16 rows of one [8, 2048]
    tile, contract with a [8,5] mask matmul whose col p selects rows
    {p, p+4} (absorbing S_a+S_b) and col 4 is all-ones (computing t
    for free) -> [5, 2048] PSUM; row-norms^2 via ACT+DVE column split;
    (-3,-3,-3,-3,1) combine matmul -> A - 3*B2; two DVE scalar ops; out.
"""

import os
import sys

sys.path.insert(0, "/opt/trn_rl_repo")
os.environ.setdefault("MYCRO_LOCAL_CACHE", "1")

import numpy as np

B, F = 8192, 8192
NPARTS = 4
D = F // NPARTS                 # 2048
NCORES = 8
B_CORE = B // NCORES            # 1024
TILE_P = 128
NTILES = B_CORE // TILE_P       # 8
MM_N = 512                      # moving free dim per matmul (PSUM bank)
NCHUNK = D // MM_N              # 4
GROWS = 2 * NPARTS              # gathered rows: AR1 block + AR2 block

_cache = {}


def _build(ncores=NCORES, dve_parts=2, dve_tiles=(NTILES - 2, NTILES - 1),
           evac_hi_eng="vector"):
    import concourse.bass as bass  # noqa: F401
    import concourse.mybir as mybir
    from concourse import bacc, tile

    f32 = mybir.dt.float32
    bf16 = mybir.dt.bfloat16
    i32 = mybir.dt.int32
    Act = mybir.ActivationFunctionType
    Alu = mybir.AluOpType
    X = mybir.AxisListType.X

    nc = bacc.Bacc("TRN2", num_devices=ncores, debug=False)
    x_t = nc.dram_tensor("x", [B_CORE, F], f32, kind="ExternalInput")
    g_t = nc.dram_tensor("gamma", [1, 1], f32, kind="ExternalInput")
    out_t = nc.dram_tensor("out", [1, 1], f32, kind="ExternalOutput")

    rg = [list(range(ncores))]
    nact = NPARTS - dve_parts

    with tile.TileContext(nc) as tc:
        with tc.tile_pool(name="xp", bufs=10) as xp, \
             tc.tile_pool(name="scratch", bufs=2) as scp, \
             tc.tile_pool(name="small", bufs=3) as stp, \
             tc.tile_pool(name="tail", bufs=1) as tlp, \
             tc.tile_pool(name="ps", bufs=1, space="PSUM") as psp, \
             tc.tile_pool(name="dram", bufs=1, space="DRAM") as dram:

            # bf16 collective path: halves evac-DMA and AR wire bytes,
            # and the post-AR reload needs no cast (precision is moot:
            # the loss is dominated by the 2P constant, quadratic terms
            # are ~1e-3 relative)
            cc_in_a = dram.tile([NPARTS, D], bf16)
            cc_out_a = dram.tile([NPARTS, D], bf16)
            cc_in_b = dram.tile([NPARTS, D], bf16)
            cc_out_b = dram.tile([NPARTS, D], bf16)

            # ---- constants (hidden under stream start) ----
            g_sb = tlp.tile([1, 1], f32, tag="g_sb")
            nc.sync.dma_start(g_sb[:], g_t[:])

            # masks [8, 5] bf16: col p selects rows with (r mod 4)==p,
            # col 4 all-ones. (partition-sliced memsets at base>0
            # mislower; build from iota + compares instead)
            idx32 = tlp.tile([GROWS, 1], i32, tag="idx32")
            nc.gpsimd.iota(idx32[:], pattern=[[0, 1]], base=0,
                           channel_multiplier=1)
            idx4 = tlp.tile([GROWS, 1], i32, tag="idx4")
            nc.vector.tensor_scalar(out=idx4[:], in0=idx32[:],
                                    scalar1=3, scalar2=None,
                                    op0=Alu.bitwise_and)
            masks = tlp.tile([GROWS, NPARTS + 1], bf16, tag="masks")
            for p in range(NPARTS):
                nc.vector.tensor_scalar(
                    out=masks[:, p:p + 1], in0=idx4[:],
                    scalar1=p, scalar2=None, op0=Alu.is_equal)
            nc.vector.memset(masks[:, NPARTS:NPARTS + 1], 1.0)
            # combine vector (-3,-3,-3,-3,1): (idx<4) * -4 + 1
            idx5 = tlp.tile([NPARTS + 1, 1], i32, tag="idx5")
            nc.gpsimd.iota(idx5[:], pattern=[[0, 1]], base=0,
                           channel_multiplier=1)
            comb = tlp.tile([NPARTS + 1, 1], f32, tag="comb")
            nc.vector.tensor_scalar(out=comb[:], in0=idx5[:],
                                    scalar1=4, scalar2=None, op0=Alu.is_lt)
            nc.vector.tensor_scalar(out=comb[:], in0=comb[:],
                                    scalar1=-4.0, scalar2=1.0,
                                    op0=Alu.mult, op1=Alu.add)

            # PSUM: part p at partition 32p (PE col tile_position).
            # S_a = tile 0 (AllReduce'd mid-stream), S_b = tiles 1..7.
            S_a = psp.tile([TILE_P, D], f32, tag="accA")
            S_b = psp.tile([TILE_P, D], f32, tag="accB")

            def evac_and_send(S_ps, cc_in, lo_tag, hi_tag):
                # evac PSUM -> SBUF full-width (junk rows harmless); two
                # SEPARATE tiles on two engines, then strided-AP DMAs
                # gather rows {0,32,64,96}
                lo = tlp.tile([TILE_P, D // 2], bf16, tag=lo_tag,
                              name=lo_tag)
                hi = tlp.tile([TILE_P, D // 2], bf16, tag=hi_tag,
                              name=hi_tag)
                nc.scalar.copy(lo[:], S_ps[:, :D // 2])
                hi_eng = getattr(nc, evac_hi_eng)
                hi_eng.tensor_copy(hi[:], S_ps[:, D // 2:])
                nc.sync.dma_start(cc_in[:, :D // 2], lo[0:TILE_P:32, :])
                nc.scalar.dma_start(cc_in[:, D // 2:], hi[0:TILE_P:32, :])

            for i in range(NTILES):
                split = i in (0, NTILES - 1)
                rows = x_t[i * TILE_P:(i + 1) * TILE_P, :]
                # SWDGE DMA casts fp32 -> bf16 in flight. Whole-tile
                # transfers keep 32KB-contiguous HBM reads; tiles 0 and 7
                # are split per part so their chains start at the first
                # part boundary (tile 0 feeds AR1 early; tile 7 is the
                # exposed tail).
                xtile = xp.tile([TILE_P, F], bf16, tag="xt")
                if split:
                    for p in range(NPARTS):
                        nc.gpsimd.dma_start(xtile[:, p * D:(p + 1) * D],
                                            rows[:, p * D:(p + 1) * D])
                else:
                    nc.gpsimd.dma_start(xtile[:], rows)
                xt = [xtile[:, p * D:(p + 1) * D] for p in range(NPARTS)]

                # per-row sum of squares: ACT parts, then DVE parts.
                # DVE only helps on the trailing tiles (shortens the
                # exposed last-tile chain); running it all-stream risks
                # 2X_2PORT SBUF-port locks starving SWDGE mid-stream.
                na = nact if i in dve_tiles else NPARTS
                ss_a = stp.tile([TILE_P, NPARTS], f32, tag="ss_a")
                ss_d = stp.tile([TILE_P, max(dve_parts, 1)], f32,
                                tag="ss_d")
                sqa = scp.tile([TILE_P, D], bf16, tag="sqa")
                for p in range(na):
                    nc.scalar.activation(
                        sqa[:], xt[p], Act.Square,
                        accum_out=ss_a[:, p:p + 1])
                for p in range(na, NPARTS):
                    sqd = scp.tile([TILE_P, D], bf16, tag="sqd")
                    nc.vector.tensor_mul(sqd[:], xt[p], xt[p])
                    nc.vector.tensor_reduce(
                        ss_d[:, p - na:p - na + 1], sqd[:],
                        axis=X, op=Alu.add)

                # norm -> reciprocal -> bf16 per engine-group
                norm_a = stp.tile([TILE_P, NPARTS], f32, tag="norm_a")
                norm_d = stp.tile([TILE_P, max(dve_parts, 1)], f32,
                                  tag="norm_d")
                r_a = stp.tile([TILE_P, NPARTS], f32, tag="r_a")
                r_d = stp.tile([TILE_P, max(dve_parts, 1)], f32,
                               tag="r_d")
                rb_a = stp.tile([TILE_P, NPARTS], bf16, tag="rb_a")
                rb_d = stp.tile([TILE_P, max(dve_parts, 1)], bf16,
                                tag="rb_d")
                if na:
                    nc.scalar.sqrt(norm_a[:, :na], ss_a[:, :na])
                    nc.vector.reciprocal(r_a[:, :na], norm_a[:, :na])
                    nc.vector.tensor_copy(rb_a[:, :na], r_a[:, :na])
                if na < NPARTS:
                    nd = NPARTS - na
                    nc.scalar.sqrt(norm_d[:, :nd], ss_d[:, :nd])
                    nc.vector.reciprocal(r_d[:, :nd], norm_d[:, :nd])
                    nc.vector.tensor_copy(rb_d[:, :nd], r_d[:, :nd])
                rb = {p: (rb_a[:, p:p + 1] if p < na
                          else rb_d[:, p - na:p - na + 1])
                      for p in range(NPARTS)}

                S_ps = S_a if i == 0 else S_b
                for p in range(NPARTS):
                    for j in range(NCHUNK):
                        nc.tensor.matmul(
                            S_ps[32 * p:32 * p + 1,
                                 j * MM_N:(j + 1) * MM_N],
                            lhsT=rb[p],
                            rhs=xtile[:, p * D + j * MM_N:
                                      p * D + (j + 1) * MM_N],
                            start=(i in (0, 1)),
                            stop=(i in (0, NTILES - 1)),
                            tile_position=(0, 32 * p))

                if i == 0:
                    # ship tile-0 partials + AllReduce now: overlapped
                    # with the DMA stream, it also backpressures fast
                    # ranks (SDMA sharing) so stream-end skew shrinks
                    evac_and_send(S_a, cc_in_a, "s_a_lo", "s_a_hi")
                    nc.gpsimd.collective_compute(
                        "AllReduce", Alu.add, replica_groups=rg,
                        ins=[cc_in_a.opt()], outs=[cc_out_a.opt()])

            # ---- endgame ----
            evac_and_send(S_b, cc_in_b, "s_b_lo", "s_b_hi")
            nc.gpsimd.collective_compute(
                "AllReduce", Alu.add, replica_groups=rg,
                ins=[cc_in_b.opt()], outs=[cc_out_b.opt()])

            # load both AR outputs as rows of one [8, 2048] tile; the
            # AR1 half loads mid-stream (hidden); the AR2 half is on the
            # critical path -> HWDGE (sync) for its lower first-byte
            # latency (no cast needed now)
            ag_sb = tlp.tile([GROWS, D], bf16, tag="ag_sb")
            nc.gpsimd.dma_start(ag_sb[0:NPARTS, :], cc_out_a[:])
            nc.sync.dma_start(ag_sb[NPARTS:GROWS, :], cc_out_b[:])

            # t5[p,:] = S_a_p + S_b_p (p<4), t5[4,:] = t = sum_p S_p
            t5 = psp.tile([NPARTS + 1, D], f32, tag="accA")
            for j in range(NCHUNK):
                nc.tensor.matmul(
                    t5[:, j * MM_N:(j + 1) * MM_N],
                    lhsT=masks[:],
                    rhs=ag_sb[:, j * MM_N:(j + 1) * MM_N],
                    start=True, stop=True, tile_position=(0, 0))

            # row-wise ||.||^2 of the 5 rows (ACT reads PSUM directly;
            # DVE can't help here -- it may only read ONE input from
            # PSUM and the square needs the same PSUM operand twice)
            sq5 = tlp.tile([NPARTS + 1, D], bf16, tag="sq5")
            acc5 = tlp.tile([NPARTS + 1, 1], f32, tag="acc5")
            nc.scalar.activation(sq5[:], t5[:], Act.Square,
                                 accum_out=acc5[:])

            # A - 3*B2 in one tiny fp32 matmul (reuses S_b's PSUM banks)
            ba = psp.tile([1, 1], f32, tag="accB")
            nc.tensor.matmul(ba[:], lhsT=comb[:], rhs=acc5[:],
                             start=True, stop=True)

            # loss = ((A - 3*B2) / B^2 + 2P) / 2 * gamma
            l0 = tlp.tile([1, 1], f32, tag="l0")
            nc.vector.tensor_scalar(
                out=l0[:], in0=ba[:],
                scalar1=1.0 / (2.0 * float(B) * float(B)),
                scalar2=float(NPARTS),
                op0=Alu.mult, op1=Alu.add)
            loss = tlp.tile([1, 1], f32, tag="loss")
            nc.vector.tensor_mul(loss[:], l0[:], g_sb[:])
            nc.sync.dma_start(out_t[:], loss[:])

    nc.compile()
    return nc


def _get_nc():
    if "nc" not in _cache:
        dt = os.environ.get("C3S_DVE_TILES", "6,7")
        dve_tiles = tuple(int(v) for v in dt.split(",") if v != "")
        _cache["nc"] = _build(
            dve_parts=int(os.environ.get("C3S_DVE_PARTS", "2")),
            dve_tiles=dve_tiles,
            evac_hi_eng=os.environ.get("C3S_EVAC_HI", "vector"))
    return _cache["nc"]


def kernel(x, gamma, **run_kwargs):
    from concourse import bass_utils

    x = np.ascontiguousarray(np.asarray(x, dtype=np.float32))
    gamma = np.asarray(gamma, dtype=np.float32).reshape(1, 1)
    assert x.shape == (B, F), x.shape

    nc = _get_nc()
    in_maps = [
        {"x": x[c * B_CORE:(c + 1) * B_CORE], "gamma": gamma}
        for c in range(NCORES)
    ]
    res = bass_utils.run_bass_kernel_spmd(
        nc, in_maps, core_ids=list(range(NCORES)), **run_kwargs)
    out = np.asarray(res.results[0]["out"], dtype=np.float32).reshape(1)
    if run_kwargs.get("trace"):
        _cache["last_results"] = res
    return out


# revision 43
# speedup vs baseline: 1.0011x; 1.0011x over previous
"""Trainium2 Bass kernel for nn_C3S_RegularLoss.

reference:
    xr = x.reshape(B, P, D); xn = xr / ||xr||_2(axis=-1)
    s = mean_b(xn)                     # (P, D)
    corr = s @ s.T                     # (P, P)
    loss = (sum(corr) - 3*trace(corr) + 2P) / 2 * gamma

Reformulated without the corr matrix, with S = sum_b xn (sum, not mean):
    loss = ((A - 3*B2) / B^2 + 2P) / 2 * gamma,
    A = ||t||^2, t = sum_p S_p, B2 = sum_p ||S_p||^2

Sharding: data-parallel over the batch dim, 8 cores x 1024 rows.

Per core:
  - stream x as 8 row-tiles of (128, 8192), SWDGE DMA with fp32->bf16
    cast in flight (HBM fp32 read side is the ~94us roofline); first and
    last tiles split per part so their compute chains pipeline at part
    granularity
  - per-row sum-of-squares: parts 0/1 on ACT (Square + free accum),
    parts 2/3 on DVE (tensor_mul + tensor_reduce; the fused
    tensor_tensor_reduce wedges this runtime) -- neither engine
    saturates, so the final tile's chain is short
  - S_p = sum_b x_b,p / ||x_b,p|| via PE: stationary r = 1/norm
    [128,1] per part, PSUM accumulation (part p at partition 32p).
    Two accumulators: S_a = tile 0 only, AllReduce'd mid-stream (the
    in-flight collective also equalizes per-rank stream pace -- an
    input-independent warmup does not, and the final AR then eats
    ~10us of rank skew); S_b = tiles 1..7, AllReduce'd at the end.
  - endgame: evac S_b (ACT/DVE halves into separate tiles), 2 DMAs ->
    cc_in_b, AR2, load both AR outputs as bf16 rows of one [8, 2048]
    tile, contract with a [8,5] mask matmul whose col p selects rows
    {p, p+4} (absorbing S_a+S_b) and col 4 is all-ones (computing t
    for free) -> [5, 2048] PSUM; row-norms^2 via ACT+DVE column split;
    (-3,-3,-3,-3,1) combine matmul -> A - 3*B2; two DVE scalar ops; out.
"""

import os
import sys

sys.path.insert(0, "/opt/trn_rl_repo")
os.environ.setdefault("MYCRO_LOCAL_CACHE", "1")

import numpy as np

B, F = 8192, 8192
NPARTS = 4
D = F // NPARTS                 # 2048
NCORES = 8
B_CORE = B // NCORES            # 1024
TILE_P = 128
NTILES = B_CORE // TILE_P       # 8
MM_N = 512                      # moving free dim per matmul (PSUM bank)
NCHUNK = D // MM_N              # 4
GROWS = 2 * NPARTS              # gathered rows: AR1 block + AR2 block

_cache = {}


def _build(ncores=NCORES, dve_parts=2, dve_tiles=(NTILES - 2, NTILES - 1),
           evac_hi_eng="vector"):
    import concourse.bass as bass  # noqa: F401
    import concourse.mybir as mybir
    from concourse import bacc, tile

    f32 = mybir.dt.float32
    bf16 = mybir.dt.bfloat16
    i32 = mybir.dt.int32
    Act = mybir.ActivationFunctionType
    Alu = mybir.AluOpType
    X = mybir.AxisListType.X

    nc = bacc.Bacc("TRN2", num_devices=ncores, debug=False)
    x_t = nc.dram_tensor("x", [B_CORE, F], f32, kind="ExternalInput")
    g_t = nc.dram_tensor("gamma", [1, 1], f32, kind="ExternalInput")
    out_t = nc.dram_tensor("out", [1, 1], f32, kind="ExternalOutput")

    rg = [list(range(ncores))]
    nact = NPARTS - dve_parts

    with tile.TileContext(nc) as tc:
        with tc.tile_pool(name="xp", bufs=10) as xp, \
             tc.tile_pool(name="scratch", bufs=2) as scp, \
             tc.tile_pool(name="small", bufs=3) as stp, \
             tc.tile_pool(name="tail", bufs=1) as tlp, \
             tc.tile_pool(name="ps", bufs=1, space="PSUM") as psp, \
             tc.tile_pool(name="dram", bufs=1, space="DRAM") as dram:

            # bf16 collective path: halves evac-DMA and AR wire bytes,
            # and the post-AR reload needs no cast (precision is moot:
            # the loss is dominated by the 2P constant, quadratic terms
            # are ~1e-3 relative)
            cc_in_a = dram.tile([NPARTS, D], bf16)
            cc_out_a = dram.tile([NPARTS, D], bf16)
            cc_in_b = dram.tile([NPARTS, D], bf16)
            cc_out_b = dram.tile([NPARTS, D], bf16)

            # ---- constants (hidden under stream start) ----
            g_sb = tlp.tile([1, 1], f32, tag="g_sb")
            nc.sync.dma_start(g_sb[:], g_t[:])

            # masks [8, 5] bf16: col p selects rows with (r mod 4)==p,
            # col 4 all-ones. (partition-sliced memsets at base>0
            # mislower; build from iota + compares instead)
            idx32 = tlp.tile([GROWS, 1], i32, tag="idx32")
            nc.gpsimd.iota(idx32[:], pattern=[[0, 1]], base=0,
                           channel_multiplier=1)
            idx4 = tlp.tile([GROWS, 1], i32, tag="idx4")
            nc.vector.tensor_scalar(out=idx4[:], in0=idx32[:],
                                    scalar1=3, scalar2=None,
                                    op0=Alu.bitwise_and)
            masks = tlp.tile([GROWS, NPARTS + 1], bf16, tag="masks")
            for p in range(NPARTS):
                nc.vector.tensor_scalar(
                    out=masks[:, p:p + 1], in0=idx4[:],
                    scalar1=p, scalar2=None, op0=Alu.is_equal)
            nc.vector.memset(masks[:, NPARTS:NPARTS + 1], 1.0)
            # combine vector (-3,-3,-3,-3,1): (idx<4) * -4 + 1
            idx5 = tlp.tile([NPARTS + 1, 1], i32, tag="idx5")
            nc.gpsimd.iota(idx5[:], pattern=[[0, 1]], base=0,
                           channel_multiplier=1)
            comb = tlp.tile([NPARTS + 1, 1], f32, tag="comb")
            nc.vector.tensor_scalar(out=comb[:], in0=idx5[:],
                                    scalar1=4, scalar2=None, op0=Alu.is_lt)
            nc.vector.tensor_scalar(out=comb[:], in0=comb[:],
                                    scalar1=-4.0, scalar2=1.0,
                                    op0=Alu.mult, op1=Alu.add)

            # PSUM: part p at partition 32p (PE col tile_position).
            # S_a = tile 0 (AllReduce'd mid-stream), S_b = tiles 1..7.
            S_a = psp.tile([TILE_P, D], f32, tag="accA")
            S_b = psp.tile([TILE_P, D], f32, tag="accB")

            def evac_and_send(S_ps, cc_in, lo_tag, hi_tag):
                # evac PSUM -> SBUF full-width (junk rows harmless); two
                # SEPARATE tiles on two engines, then strided-AP DMAs
                # gather rows {0,32,64,96}
                lo = tlp.tile([TILE_P, D // 2], bf16, tag=lo_tag,
                              name=lo_tag)
                hi = tlp.tile([TILE_P, D // 2], bf16, tag=hi_tag,
                              name=hi_tag)
                hi_eng = getattr(nc, evac_hi_eng)
                hi_eng.tensor_copy(hi[:], S_ps[:, D // 2:])
                nc.scalar.copy(lo[:], S_ps[:, :D // 2])
                nc.sync.dma_start(cc_in[:, :D // 2], lo[0:TILE_P:32, :])
                nc.scalar.dma_start(cc_in[:, D // 2:], hi[0:TILE_P:32, :])

            for i in range(NTILES):
                split = i in (0, NTILES - 1)
                rows = x_t[i * TILE_P:(i + 1) * TILE_P, :]
                # SWDGE DMA casts fp32 -> bf16 in flight. Whole-tile
                # transfers keep 32KB-contiguous HBM reads; tiles 0 and 7
                # are split per part so their chains start at the first
                # part boundary (tile 0 feeds AR1 early; tile 7 is the
                # exposed tail).
                xtile = xp.tile([TILE_P, F], bf16, tag="xt")
                if split:
                    for p in range(NPARTS):
                        nc.gpsimd.dma_start(xtile[:, p * D:(p + 1) * D],
                                            rows[:, p * D:(p + 1) * D])
                else:
                    nc.gpsimd.dma_start(xtile[:], rows)
                xt = [xtile[:, p * D:(p + 1) * D] for p in range(NPARTS)]

                # per-row sum of squares: ACT parts, then DVE parts.
                # DVE only helps on the trailing tiles (shortens the
                # exposed last-tile chain); running it all-stream risks
                # 2X_2PORT SBUF-port locks starving SWDGE mid-stream.
                na = nact if i in dve_tiles else NPARTS
                ss_a = stp.tile([TILE_P, NPARTS], f32, tag="ss_a")
                ss_d = stp.tile([TILE_P, max(dve_parts, 1)], f32,
                                tag="ss_d")
                sqa = scp.tile([TILE_P, D], bf16, tag="sqa")
                for p in range(na):
                    nc.scalar.activation(
                        sqa[:], xt[p], Act.Square,
                        accum_out=ss_a[:, p:p + 1])
                for p in range(na, NPARTS):
                    sqd = scp.tile([TILE_P, D], bf16, tag="sqd")
                    nc.vector.tensor_mul(sqd[:], xt[p], xt[p])
                    nc.vector.tensor_reduce(
                        ss_d[:, p - na:p - na + 1], sqd[:],
                        axis=X, op=Alu.add)

                # norm -> reciprocal -> bf16 per engine-group
                norm_a = stp.tile([TILE_P, NPARTS], f32, tag="norm_a")
                norm_d = stp.tile([TILE_P, max(dve_parts, 1)], f32,
                                  tag="norm_d")
                r_a = stp.tile([TILE_P, NPARTS], f32, tag="r_a")
                r_d = stp.tile([TILE_P, max(dve_parts, 1)], f32,
                               tag="r_d")
                rb_a = stp.tile([TILE_P, NPARTS], bf16, tag="rb_a")
                rb_d = stp.tile([TILE_P, max(dve_parts, 1)], bf16,
                                tag="rb_d")
                if na:
                    nc.scalar.sqrt(norm_a[:, :na], ss_a[:, :na])
                    nc.vector.reciprocal(r_a[:, :na], norm_a[:, :na])
                    nc.vector.tensor_copy(rb_a[:, :na], r_a[:, :na])
                if na < NPARTS:
                    nd = NPARTS - na
                    nc.scalar.sqrt(norm_d[:, :nd], ss_d[:, :nd])
                    nc.vector.reciprocal(r_d[:, :nd], norm_d[:, :nd])
                    nc.vector.tensor_copy(rb_d[:, :nd], r_d[:, :nd])
                rb = {p: (rb_a[:, p:p + 1] if p < na
                          else rb_d[:, p - na:p - na + 1])
                      for p in range(NPARTS)}

                S_ps = S_a if i == 0 else S_b
                for p in range(NPARTS):
                    for j in range(NCHUNK):
                        nc.tensor.matmul(
                            S_ps[32 * p:32 * p + 1,
                                 j * MM_N:(j + 1) * MM_N],
                            lhsT=rb[p],
                            rhs=xtile[:, p * D + j * MM_N:
                                      p * D + (j + 1) * MM_N],
                            start=(i in (0, 1)),
                            stop=(i in (0, NTILES - 1)),
                            tile_position=(0, 32 * p))

                if i == 0:
                    # ship tile-0 partials + AllReduce now: overlapped
                    # with the DMA stream, it also backpressures fast
                    # ranks (SDMA sharing) so stream-end skew shrinks
                    evac_and_send(S_a, cc_in_a, "s_a_lo", "s_a_hi")
                    nc.gpsimd.collective_compute(
                        "AllReduce", Alu.add, replica_groups=rg,
                        ins=[cc_in_a.opt()], outs=[cc_out_a.opt()])

            # ---- endgame ----
            evac_and_send(S_b, cc_in_b, "s_b_lo", "s_b_hi")
            nc.gpsimd.collective_compute(
                "AllReduce", Alu.add, replica_groups=rg,
                ins=[cc_in_b.opt()], outs=[cc_out_b.opt()])

            # load both AR outputs as rows of one [8, 2048] tile, both
            # on HWDGE (no cast needed in the bf16 cc path). The ag_a
            # load must NOT sit on the gpsimd queue: emitted there ahead
            # of the AR2 doorbell, its wait on AR1-completion (which
            # ncfw runs lazily near stream end) holds the doorbell
            # hostage for ~3us.
            ag_sb = tlp.tile([GROWS, D], bf16, tag="ag_sb")
            nc.sync.dma_start(ag_sb[0:NPARTS, :], cc_out_a[:])
            nc.sync.dma_start(ag_sb[NPARTS:GROWS, :], cc_out_b[:])

            # t5[p,:] = S_a_p + S_b_p (p<4), t5[4,:] = t = sum_p S_p
            t5 = psp.tile([NPARTS + 1, D], f32, tag="accA")
            for j in range(NCHUNK):
                nc.tensor.matmul(
                    t5[:, j * MM_N:(j + 1) * MM_N],
                    lhsT=masks[:],
                    rhs=ag_sb[:, j * MM_N:(j + 1) * MM_N],
                    start=True, stop=True, tile_position=(0, 0))

            # row-wise ||.||^2 of the 5 rows (ACT reads PSUM directly;
            # DVE can't help here -- it may only read ONE input from
            # PSUM and the square needs the same PSUM operand twice)
            sq5 = tlp.tile([NPARTS + 1, D], bf16, tag="sq5")
            acc5 = tlp.tile([NPARTS + 1, 1], f32, tag="acc5")
            nc.scalar.activation(sq5[:], t5[:], Act.Square,
                                 accum_out=acc5[:])

            # A - 3*B2 in one tiny fp32 matmul (reuses S_b's PSUM banks)
            ba = psp.tile([1, 1], f32, tag="accB")
            nc.tensor.matmul(ba[:], lhsT=comb[:], rhs=acc5[:],
                             start=True, stop=True)

            # loss = ((A - 3*B2) / B^2 + 2P) / 2 * gamma
            l0 = tlp.tile([1, 1], f32, tag="l0")
            nc.vector.tensor_scalar(
                out=l0[:], in0=ba[:],
                scalar1=1.0 / (2.0 * float(B) * float(B)),
                scalar2=float(NPARTS),
                op0=Alu.mult, op1=Alu.add)
            loss = tlp.tile([1, 1], f32, tag="loss")
            nc.vector.tensor_mul(loss[:], l0[:], g_sb[:])
            nc.sync.dma_start(out_t[:], loss[:])

    nc.compile()
    return nc


def _get_nc():
    if "nc" not in _cache:
        dt = os.environ.get("C3S_DVE_TILES", "6,7")
        dve_tiles = tuple(int(v) for v in dt.split(",") if v != "")
        _cache["nc"] = _build(
            dve_parts=int(os.environ.get("C3S_DVE_PARTS", "2")),
            dve_tiles=dve_tiles,
            evac_hi_eng=os.environ.get("C3S_EVAC_HI", "vector"))
    return _cache["nc"]


def kernel(x, gamma, **run_kwargs):
    from concourse import bass_utils

    x = np.ascontiguousarray(np.asarray(x, dtype=np.float32))
    gamma = np.asarray(gamma, dtype=np.float32).reshape(1, 1)
    assert x.shape == (B, F), x.shape

    nc = _get_nc()
    in_maps = [
        {"x": x[c * B_CORE:(c + 1) * B_CORE], "gamma": gamma}
        for c in range(NCORES)
    ]
    res = bass_utils.run_bass_kernel_spmd(
        nc, in_maps, core_ids=list(range(NCORES)), **run_kwargs)
    out = np.asarray(res.results[0]["out"], dtype=np.float32).reshape(1)
    if run_kwargs.get("trace"):
        _cache["last_results"] = res
    return out


# revision 45
# speedup vs baseline: 1.0492x; 1.0480x over previous
"""Trainium2 Bass kernel for nn_C3S_RegularLoss.

reference:
    xr = x.reshape(B, P, D); xn = xr / ||xr||_2(axis=-1)
    s = mean_b(xn)                     # (P, D)
    corr = s @ s.T                     # (P, P)
    loss = (sum(corr) - 3*trace(corr) + 2P) / 2 * gamma

Reformulated without the corr matrix, with S = sum_b xn (sum, not mean):
    loss = ((A - 3*B2) / B^2 + 2P) / 2 * gamma,
    A = ||t||^2, t = sum_p S_p, B2 = sum_p ||S_p||^2

Sharding: data-parallel over the batch dim, 8 cores x 1024 rows.

Per core:
  - stream x as 8 row-tiles of (128, 8192), SWDGE DMA with fp32->bf16
    cast in flight (HBM fp32 read side is the ~94us roofline); first and
    last tiles split per part so their compute chains pipeline at part
    granularity
  - per-row sum-of-squares: parts 0/1 on ACT (Square + free accum),
    parts 2/3 on DVE (tensor_mul + tensor_reduce; the fused
    tensor_tensor_reduce wedges this runtime) -- neither engine
    saturates, so the final tile's chain is short
  - S_p = sum_b x_b,p / ||x_b,p|| via PE: stationary r = 1/norm
    [128,1] per part, PSUM accumulation (part p at partition 32p).
    Two accumulators: S_a = tile 0 only, AllReduce'd mid-stream (the
    in-flight collective also equalizes per-rank stream pace -- an
    input-independent warmup does not, and the final AR then eats
    ~10us of rank skew); S_b = tiles 1..7, AllReduce'd at the end.
  - endgame: evac S_b (ACT/DVE halves into separate tiles), 2 DMAs ->
    cc_in_b, AR2, load both AR outputs as bf16 rows of one [8, 2048]
    tile, contract with a [8,5] mask matmul whose col p selects rows
    {p, p+4} (absorbing S_a+S_b) and col 4 is all-ones (computing t
    for free) -> [5, 2048] PSUM; row-norms^2 via ACT+DVE column split;
    (-3,-3,-3,-3,1) combine matmul -> A - 3*B2; two DVE scalar ops; out.
"""

import os
import sys

sys.path.insert(0, "/opt/trn_rl_repo")
os.environ.setdefault("MYCRO_LOCAL_CACHE", "1")

import numpy as np

B, F = 8192, 8192
NPARTS = 4
D = F // NPARTS                 # 2048
NCORES = 8
B_CORE = B // NCORES            # 1024
TILE_P = 128
NTILES = B_CORE // TILE_P       # 8
MM_N = 512                      # moving free dim per matmul (PSUM bank)
NCHUNK = D // MM_N              # 4
GROWS = 2 * NPARTS              # gathered rows: AR1 block + AR2 block

_cache = {}


def _build(ncores=NCORES, dve_parts=2, dve_tiles=(NTILES - 2, NTILES - 1),
           evac_hi_eng="vector"):
    import concourse.bass as bass  # noqa: F401
    import concourse.mybir as mybir
    from concourse import bacc, tile

    f32 = mybir.dt.float32
    bf16 = mybir.dt.bfloat16
    i32 = mybir.dt.int32
    Act = mybir.ActivationFunctionType
    Alu = mybir.AluOpType
    X = mybir.AxisListType.X

    nc = bacc.Bacc("TRN2", num_devices=ncores, debug=False)
    x_t = nc.dram_tensor("x", [B_CORE, F], f32, kind="ExternalInput")
    g_t = nc.dram_tensor("gamma", [1, 1], f32, kind="ExternalInput")
    out_t = nc.dram_tensor("out", [1, 1], f32, kind="ExternalOutput")

    rg = [list(range(ncores))]
    nact = NPARTS - dve_parts

    with tile.TileContext(nc) as tc:
        with tc.tile_pool(name="xp", bufs=10) as xp, \
             tc.tile_pool(name="scratch", bufs=2) as scp, \
             tc.tile_pool(name="small", bufs=3) as stp, \
             tc.tile_pool(name="tail", bufs=1) as tlp, \
             tc.tile_pool(name="ps", bufs=1, space="PSUM") as psp, \
             tc.tile_pool(name="dram", bufs=1, space="DRAM") as dram:

            # bf16 collective path: halves evac-DMA and AR wire bytes,
            # and the post-AR reload needs no cast (precision is moot:
            # the loss is dominated by the 2P constant, quadratic terms
            # are ~1e-3 relative)
            cc_in_a = dram.tile([NPARTS, D], bf16)
            cc_out_a = dram.tile([NPARTS, D], bf16)
            cc_in_b = dram.tile([NPARTS, D], bf16)
            cc_out_b = dram.tile([NPARTS, D], bf16)

            # ---- constants (hidden under stream start) ----
            g_sb = tlp.tile([1, 1], f32, tag="g_sb")
            nc.sync.dma_start(g_sb[:], g_t[:])

            # masks [8, 5] bf16: col p selects rows with (r mod 4)==p,
            # col 4 all-ones. (partition-sliced memsets at base>0
            # mislower; build from iota + compares instead)
            idx32 = tlp.tile([GROWS, 1], i32, tag="idx32")
            nc.gpsimd.iota(idx32[:], pattern=[[0, 1]], base=0,
                           channel_multiplier=1)
            idx4 = tlp.tile([GROWS, 1], i32, tag="idx4")
            nc.vector.tensor_scalar(out=idx4[:], in0=idx32[:],
                                    scalar1=3, scalar2=None,
                                    op0=Alu.bitwise_and)
            masks = tlp.tile([GROWS, NPARTS + 1], bf16, tag="masks")
            for p in range(NPARTS):
                nc.vector.tensor_scalar(
                    out=masks[:, p:p + 1], in0=idx4[:],
                    scalar1=p, scalar2=None, op0=Alu.is_equal)
            nc.vector.memset(masks[:, NPARTS:NPARTS + 1], 1.0)
            # combine vector (-3,-3,-3,-3,1): (idx<4) * -4 + 1
            idx5 = tlp.tile([NPARTS + 1, 1], i32, tag="idx5")
            nc.gpsimd.iota(idx5[:], pattern=[[0, 1]], base=0,
                           channel_multiplier=1)
            comb = tlp.tile([NPARTS + 1, 1], f32, tag="comb")
            nc.vector.tensor_scalar(out=comb[:], in0=idx5[:],
                                    scalar1=4, scalar2=None, op0=Alu.is_lt)
            nc.vector.tensor_scalar(out=comb[:], in0=comb[:],
                                    scalar1=-4.0, scalar2=1.0,
                                    op0=Alu.mult, op1=Alu.add)

            # PSUM: part p at partition 32p (PE col tile_position).
            # S_a = tile 0 (AllReduce'd mid-stream), S_b = tiles 1..7.
            S_a = psp.tile([TILE_P, D], f32, tag="accA")
            S_b = psp.tile([TILE_P, D], f32, tag="accB")

            def evac_and_send(S_ps, cc_in, lo_tag, hi_tag):
                # evac PSUM -> SBUF full-width (junk rows harmless); two
                # SEPARATE tiles on two engines, then strided-AP DMAs
                # gather rows {0,32,64,96}
                lo = tlp.tile([TILE_P, D // 2], bf16, tag=lo_tag,
                              name=lo_tag)
                hi = tlp.tile([TILE_P, D // 2], bf16, tag=hi_tag,
                              name=hi_tag)
                hi_eng = getattr(nc, evac_hi_eng)
                hi_eng.tensor_copy(hi[:], S_ps[:, D // 2:])
                nc.scalar.copy(lo[:], S_ps[:, :D // 2])
                nc.sync.dma_start(cc_in[:, :D // 2], lo[0:TILE_P:32, :])
                nc.scalar.dma_start(cc_in[:, D // 2:], hi[0:TILE_P:32, :])

            for i in range(NTILES):
                split = i in (0, NTILES - 1)
                rows = x_t[i * TILE_P:(i + 1) * TILE_P, :]
                # SWDGE DMA casts fp32 -> bf16 in flight. Whole-tile
                # transfers keep 32KB-contiguous HBM reads; tiles 0 and 7
                # are split per part so their chains start at the first
                # part boundary (tile 0 feeds AR1 early; tile 7 is the
                # exposed tail).
                xtile = xp.tile([TILE_P, F], bf16, tag="xt")
                if split:
                    for p in range(NPARTS):
                        nc.gpsimd.dma_start(xtile[:, p * D:(p + 1) * D],
                                            rows[:, p * D:(p + 1) * D])
                else:
                    nc.gpsimd.dma_start(xtile[:], rows)
                xt = [xtile[:, p * D:(p + 1) * D] for p in range(NPARTS)]

                # per-row sum of squares: ACT parts, then DVE parts.
                # DVE only helps on the trailing tiles (shortens the
                # exposed last-tile chain); running it all-stream risks
                # 2X_2PORT SBUF-port locks starving SWDGE mid-stream.
                na = nact if i in dve_tiles else NPARTS
                ss_a = stp.tile([TILE_P, NPARTS], f32, tag="ss_a")
                ss_d = stp.tile([TILE_P, max(dve_parts, 1)], f32,
                                tag="ss_d")
                sqa = scp.tile([TILE_P, D], bf16, tag="sqa")
                for p in range(na):
                    nc.scalar.activation(
                        sqa[:], xt[p], Act.Square,
                        accum_out=ss_a[:, p:p + 1])
                for p in range(na, NPARTS):
                    sqd = scp.tile([TILE_P, D], bf16, tag="sqd")
                    nc.vector.tensor_mul(sqd[:], xt[p], xt[p])
                    nc.vector.tensor_reduce(
                        ss_d[:, p - na:p - na + 1], sqd[:],
                        axis=X, op=Alu.add)

                # norm -> reciprocal -> bf16 per engine-group
                norm_a = stp.tile([TILE_P, NPARTS], f32, tag="norm_a")
                norm_d = stp.tile([TILE_P, max(dve_parts, 1)], f32,
                                  tag="norm_d")
                r_a = stp.tile([TILE_P, NPARTS], f32, tag="r_a")
                r_d = stp.tile([TILE_P, max(dve_parts, 1)], f32,
                               tag="r_d")
                rb_a = stp.tile([TILE_P, NPARTS], bf16, tag="rb_a")
                rb_d = stp.tile([TILE_P, max(dve_parts, 1)], bf16,
                                tag="rb_d")
                if na:
                    nc.scalar.sqrt(norm_a[:, :na], ss_a[:, :na])
                    nc.vector.reciprocal(r_a[:, :na], norm_a[:, :na])
                    nc.vector.tensor_copy(rb_a[:, :na], r_a[:, :na])
                if na < NPARTS:
                    nd = NPARTS - na
                    nc.scalar.sqrt(norm_d[:, :nd], ss_d[:, :nd])
                    nc.vector.reciprocal(r_d[:, :nd], norm_d[:, :nd])
                    nc.vector.tensor_copy(rb_d[:, :nd], r_d[:, :nd])
                rb = {p: (rb_a[:, p:p + 1] if p < na
                          else rb_d[:, p - na:p - na + 1])
                      for p in range(NPARTS)}

                S_ps = S_a if i == 0 else S_b
                for p in range(NPARTS):
                    for j in range(NCHUNK):
                        nc.tensor.matmul(
                            S_ps[32 * p:32 * p + 1,
                                 j * MM_N:(j + 1) * MM_N],
                            lhsT=rb[p],
                            rhs=xtile[:, p * D + j * MM_N:
                                      p * D + (j + 1) * MM_N],
                            start=(i in (0, 1)),
                            stop=(i in (0, NTILES - 1)),
                            tile_position=(0, 32 * p))

                if i == 0:
                    # ship tile-0 partials + AllReduce now: overlapped
                    # with the DMA stream, it also backpressures fast
                    # ranks (SDMA sharing) so stream-end skew shrinks
                    evac_and_send(S_a, cc_in_a, "s_a_lo", "s_a_hi")
                    nc.gpsimd.collective_compute(
                        "AllReduce", Alu.add, replica_groups=rg,
                        ins=[cc_in_a.opt()], outs=[cc_out_a.opt()])

            # ---- endgame ----
            evac_and_send(S_b, cc_in_b, "s_b_lo", "s_b_hi")
            ar2 = nc.gpsimd.collective_compute(
                "AllReduce", Alu.add, replica_groups=rg,
                ins=[cc_in_b.opt()], outs=[cc_out_b.opt()])

            # load both AR outputs as rows of one [8, 2048] tile, both
            # on HWDGE (no cast needed in the bf16 cc path). The ag_a
            # load must NOT sit on the gpsimd queue: emitted there ahead
            # of the AR2 doorbell, its wait on AR1-completion (which
            # ncfw runs lazily near stream end) holds the doorbell
            # hostage for ~3us.
            ag_sb = tlp.tile([GROWS, D], bf16, tag="ag_sb")
            ld_a = nc.sync.dma_start(ag_sb[0:NPARTS, :], cc_out_a[:])
            # pin scheduling: the ag_a load waits on AR1 (which ncfw
            # runs lazily near stream end); without this hint the
            # scheduler hoists it ahead of cc_in_b's DMA on the sync
            # queue and the AR2 doorbell is held hostage by AR1
            tile.add_dep_helper(ld_a.ins, ar2.ins, sync=False,
                                reason="AR2 doorbell before ag_a load")
            nc.sync.dma_start(ag_sb[NPARTS:GROWS, :], cc_out_b[:])

            # t5[p,:] = S_a_p + S_b_p (p<4), t5[4,:] = t = sum_p S_p
            t5 = psp.tile([NPARTS + 1, D], f32, tag="accA")
            for j in range(NCHUNK):
                nc.tensor.matmul(
                    t5[:, j * MM_N:(j + 1) * MM_N],
                    lhsT=masks[:],
                    rhs=ag_sb[:, j * MM_N:(j + 1) * MM_N],
                    start=True, stop=True, tile_position=(0, 0))

            # row-wise ||.||^2 of the 5 rows (ACT reads PSUM directly;
            # DVE can't help here -- it may only read ONE input from
            # PSUM and the square needs the same PSUM operand twice)
            sq5 = tlp.tile([NPARTS + 1, D], bf16, tag="sq5")
            acc5 = tlp.tile([NPARTS + 1, 1], f32, tag="acc5")
            nc.scalar.activation(sq5[:], t5[:], Act.Square,
                                 accum_out=acc5[:])

            # A - 3*B2 in one tiny fp32 matmul (reuses S_b's PSUM banks)
            ba = psp.tile([1, 1], f32, tag="accB")
            nc.tensor.matmul(ba[:], lhsT=comb[:], rhs=acc5[:],
                             start=True, stop=True)

            # loss = ((A - 3*B2) / B^2 + 2P) / 2 * gamma
            l0 = tlp.tile([1, 1], f32, tag="l0")
            nc.vector.tensor_scalar(
                out=l0[:], in0=ba[:],
                scalar1=1.0 / (2.0 * float(B) * float(B)),
                scalar2=float(NPARTS),
                op0=Alu.mult, op1=Alu.add)
            loss = tlp.tile([1, 1], f32, tag="loss")
            nc.vector.tensor_mul(loss[:], l0[:], g_sb[:])
            nc.sync.dma_start(out_t[:], loss[:])

    nc.compile()
    return nc


def _get_nc():
    if "nc" not in _cache:
        dt = os.environ.get("C3S_DVE_TILES", "6,7")
        dve_tiles = tuple(int(v) for v in dt.split(",") if v != "")
        _cache["nc"] = _build(
            dve_parts=int(os.environ.get("C3S_DVE_PARTS", "2")),
            dve_tiles=dve_tiles,
            evac_hi_eng=os.environ.get("C3S_EVAC_HI", "vector"))
    return _cache["nc"]


def kernel(x, gamma, **run_kwargs):
    from concourse import bass_utils

    x = np.ascontiguousarray(np.asarray(x, dtype=np.float32))
    gamma = np.asarray(gamma, dtype=np.float32).reshape(1, 1)
    assert x.shape == (B, F), x.shape

    nc = _get_nc()
    in_maps = [
        {"x": x[c * B_CORE:(c + 1) * B_CORE], "gamma": gamma}
        for c in range(NCORES)
    ]
    res = bass_utils.run_bass_kernel_spmd(
        nc, in_maps, core_ids=list(range(NCORES)), **run_kwargs)
    out = np.asarray(res.results[0]["out"], dtype=np.float32).reshape(1)
    if run_kwargs.get("trace"):
        _cache["last_results"] = res
    return out
